# revision 2
# baseline (speedup 1.0000x reference)
"""GNN message-passing kernel v2 — bf16, balanced variable-block cells.

Differences vs baseline kernel.py:
  - All gather tables / gathered rows / one-hot / matmuls / MLP in bf16
    (psum accumulation stays fp32); final layer emits fp32.
  - Host-side 4-dim greedy balancing of nodes -> tiles so each
    (tile, dir, half) cell is ~1020 edges; per-cell block counts are
    variable (mostly 8), cutting gather descriptors ~10% vs uniform H=9.
  - Tiles dealt to cores sorted by block profile; program uses the
    position-wise max profile so one SPMD program fits all cores.
  - ce (one-hot keys + e weights) and xT preloaded to SBUF once.
  - One-hot built with two-scalar tensor_scalar (keeps DVE 2x mode).
"""

import os
import sys

sys.path.insert(0, "/opt/trn_rl_repo")

import numpy as np
import ml_dtypes

from concourse import bass, bacc, mybir, tile
from concourse import bass_utils

N = 50000
E = 800000
D = 128
N_CORES = 8
T_CORE = 49
T_TOT = N_CORES * T_CORE
NPAD = T_TOT * 128
HALF = NPAD // 2
CHUNK = 8            # gather blocks per dma_gather call (single packet)
LOOKAHEAD = 3        # tiles of gather lookahead

f32 = mybir.dt.float32
bf16 = mybir.dt.bfloat16
i16 = mybir.dt.int16

LAST_RESULTS = None


def _register_ntff_hook():
    import types, ctypes, contextlib

    if "antenv.axon_hooks" in sys.modules:
        return
    so_path = "/opt/axon/libaxon_pjrt.so"
    if not os.path.exists(so_path):
        return
    try:
        lib = ctypes.CDLL(so_path)
        if not hasattr(lib, "axon_start_nrt_profile"):
            return
        lib.axon_start_nrt_profile.argtypes = [
            ctypes.POINTER(ctypes.c_int64), ctypes.c_size_t]
        lib.axon_start_nrt_profile.restype = ctypes.c_int64
        lib.axon_stop_nrt_profile.argtypes = [ctypes.c_char_p]
        lib.axon_stop_nrt_profile.restype = ctypes.c_int64

        @contextlib.contextmanager
        def _hook(output_dir, device_ids):
            import jax
            jax.devices()
            if device_ids:
                ids = (ctypes.c_int64 * len(device_ids))(*device_ids)
                rc = lib.axon_start_nrt_profile(ids, len(device_ids))
            else:
                rc = lib.axon_start_nrt_profile(None, 0)
            if rc != 0:
                raise RuntimeError(f"axon_start_nrt_profile rc={rc}")
            try:
                yield
            finally:
                n = lib.axon_stop_nrt_profile(str(output_dir).encode())
                print(f"profile: {n} file(s) -> {output_dir}", file=sys.stderr)

        mod = types.ModuleType("antenv.axon_hooks")
        mod.get_axon_ntff_profile_hook = lambda: _hook
        sys.modules["antenv.axon_hooks"] = mod
    except OSError:
        pass


def _wrap_idx(arr):
    L = arr.shape[0]
    w = arr.reshape(L // 16, 16).T
    return np.ascontiguousarray(np.tile(w, (8, 1)))


def _greedy_tiles(c):
    """Assign padded nodes to (tile, part) balancing the 4 cell loads."""
    tot = c.sum(1)
    order = np.argsort(-tot, kind="stable")
    loads = np.zeros((T_TOT, 4), np.float64)
    counts = np.zeros(T_TOT, np.int64)
    gtile = np.empty(NPAD, np.int32)
    gpart = np.empty(NPAD, np.int32)
    for u in order:
        proj = loads + c[u]
        score = proj.max(1) + 1e-3 * proj.sum(1)
        score[counts >= 128] = 1e18
        t = int(np.argmin(score))
        gtile[u] = t
        gpart[u] = counts[t]
        loads[t] += c[u]
        counts[t] += 1
    return gtile, gpart


def _preprocess(x, e, edge_index):
    src = np.asarray(edge_index[0], np.int64)
    dst = np.asarray(edge_index[1], np.int64)
    ew = np.asarray(e, np.float32)

    c = np.zeros((NPAD, 4), np.int64)
    c[:, 0] = np.bincount(dst[src < HALF], minlength=NPAD)
    c[:, 1] = np.bincount(dst[src >= HALF], minlength=NPAD)
    c[:, 2] = np.bincount(src[dst < HALF], minlength=NPAD)
    c[:, 3] = np.bincount(src[dst >= HALF], minlength=NPAD)
    gtile, gpart = _greedy_tiles(c)

    cells = np.zeros((T_TOT, 4), np.int64)
    cells[:, 0] = np.bincount(gtile[dst[src < HALF]], minlength=T_TOT)
    cells[:, 1] = np.bincount(gtile[dst[src >= HALF]], minlength=T_TOT)
    cells[:, 2] = np.bincount(gtile[src[dst < HALF]], minlength=T_TOT)
    cells[:, 3] = np.bincount(gtile[src[dst >= HALF]], minlength=T_TOT)
    blocks = np.ceil(cells / 128).astype(np.int64)

    # deal tiles to cores (snake by total blocks), sort within core by profile
    key = blocks.sum(1) * 10**9 + blocks[:, 0] * 10**6 + blocks[:, 1] * 10**3
    order = np.argsort(-key, kind="stable")
    core_tiles = [[] for _ in range(N_CORES)]
    for i, t in enumerate(order):
        r, k = divmod(i, N_CORES)
        core_tiles[k if r % 2 == 0 else N_CORES - 1 - k].append(int(t))
    for k in range(N_CORES):
        core_tiles[k].sort(key=lambda t: tuple(-blocks[t]))
    prof = np.array([[blocks[t] for t in core_tiles[k]]
                     for k in range(N_CORES)])          # [8, 49, 4]
    progmax = prof.max(0)                               # [49, 4] program profile

    # per-direction sorted edge partitions by cell = gtile(key)*2 + half(gat)
    def _sort_dir(keyn, gatn):
        cell = gtile[keyn].astype(np.int64) * 2 + (gatn >= HALF)
        o = np.argsort(cell, kind="stable")
        cnt = np.bincount(cell[o], minlength=T_TOT * 2)
        starts = np.zeros(T_TOT * 2 + 1, np.int64)
        starts[1:] = np.cumsum(cnt)
        return o, starts

    o_mi, st_mi = _sort_dir(dst, src)
    o_mo, st_mo = _sort_dir(src, dst)

    # stream sizes (uniform across cores by construction)
    dimof = {("mi", 0): 0, ("mi", 1): 1, ("mo", 0): 2, ("mo", 1): 3}
    NBLK = {s: int(progmax[:, dimof[s]].sum()) for s in dimof}
    # per-tile-position block starts per stream
    blk_start = {s: np.concatenate([[0], np.cumsum(progmax[:, dimof[s]])])
                 for s in dimof}
    # ce columns per tile position: [mi_k | mi_e | mo_k | mo_e]
    nb_mi = progmax[:, 0] + progmax[:, 1]
    nb_mo = progmax[:, 2] + progmax[:, 3]
    ce_w = 2 * (nb_mi + nb_mo)
    ce_off = np.concatenate([[0], np.cumsum(ce_w)])
    CE_COLS = int(ce_off[-1])

    perm_nodes = np.empty(NPAD, np.int64)
    gslot = gtile.astype(np.int64) * 128 + gpart
    perm_nodes[gslot] = np.arange(NPAD)
    xpad = np.zeros((NPAD, D), np.float32)
    xpad[:N] = np.asarray(x, np.float32)
    xpermT = np.ascontiguousarray(xpad[perm_nodes].T)   # [128, NPAD] f32

    per_core = []
    x_lo = np.asarray(xpad[:HALF], ml_dtypes.bfloat16)
    x_hi = np.asarray(xpad[HALF:], ml_dtypes.bfloat16)
    iota = np.broadcast_to(np.arange(128, dtype=np.float32),
                           (128, 128)).astype(ml_dtypes.bfloat16)

    for k in range(N_CORES):
        gidx = {s: np.zeros(NBLK[s] * 128, np.int16) for s in dimof}
        ce = np.zeros((128, CE_COLS), np.float32)
        for pos in range(T_CORE):
            t = core_tiles[k][pos]
            for dname, (ordx, starts, keyn, gatn) in (
                    ("mi", (o_mi, st_mi, dst, src)),
                    ("mo", (o_mo, st_mo, src, dst))):
                base = ce_off[pos] if dname == "mi" else \
                    ce_off[pos] + 2 * nb_mi[pos]
                nbt = nb_mi[pos] if dname == "mi" else nb_mo[pos]
                joff = 0
                for h in (0, 1):
                    s = (dname, h)
                    eidx = ordx[starts[t * 2 + h]:starts[t * 2 + h + 1]]
                    cnt = len(eidx)
                    nb_p = progmax[pos, dimof[s]]
                    assert cnt <= nb_p * 128
                    goff = blk_start[s][pos] * 128
                    gidx[s][goff:goff + cnt] = (
                        gatn[eidx] - h * HALF).astype(np.int16)
                    # key/e columns: block j, edge p -> [p, base + joff + j]
                    kcol = np.zeros(nb_p * 128, np.float32)
                    ecol = np.zeros(nb_p * 128, np.float32)
                    kcol[:cnt] = gpart[keyn[eidx]]
                    ecol[:cnt] = ew[eidx]
                    ce[:, base + joff:base + joff + nb_p] = \
                        kcol.reshape(nb_p, 128).T
                    ce[:, base + nbt + joff:base + nbt + joff + nb_p] = \
                        ecol.reshape(nb_p, 128).T
                    joff += nb_p
        ts = core_tiles[k]
        slot_sel = np.concatenate(
            [np.arange(t * 128, (t + 1) * 128) for t in ts])
        m = {
            "x_lo": x_lo,
            "x_hi": x_hi,
            "xT": np.ascontiguousarray(
                xpermT[:, slot_sel]).astype(ml_dtypes.bfloat16),
            "ce": np.asarray(ce, ml_dtypes.bfloat16),
            "iota": iota,
        }
        for s in dimof:
            m[f"idx_{s[0]}{s[1]}"] = _wrap_idx(gidx[s])
        per_core.append(m)

    layout = {
        "progmax": progmax, "NBLK": NBLK, "blk_start": blk_start,
        "nb_mi": nb_mi, "nb_mo": nb_mo, "ce_off": ce_off,
        "CE_COLS": CE_COLS, "dimof": dimof,
    }
    # output slot of original node n on the PERMUTED (core-dealt) grid:
    # core k, position pos, partition p  ->  global out column
    tile_to_pos = np.empty(T_TOT, np.int64)
    for k in range(N_CORES):
        for pos, t in enumerate(core_tiles[k]):
            tile_to_pos[t] = k * T_CORE + pos
    out_col = tile_to_pos[gtile] * 128 + gpart   # [NPAD]
    return per_core, out_col, layout


_NC_CACHE = {}


def _build_nc(layout):
    sig = (tuple(layout["progmax"].reshape(-1).tolist()),)
    if sig in _NC_CACHE:
        return _NC_CACHE[sig]
    progmax = layout["progmax"]
    dimof = layout["dimof"]
    NBLK = layout["NBLK"]
    blk_start = layout["blk_start"]
    nb_mi = layout["nb_mi"]
    nb_mo = layout["nb_mo"]
    ce_off = layout["ce_off"]
    CE_COLS = layout["CE_COLS"]

    nc = bacc.Bacc("TRN2", target_bir_lowering=False, debug=False,
                   enable_asserts=False, num_devices=N_CORES,
                   num_swdge_queues=4)

    x_lo = nc.dram_tensor("x_lo", [HALF, D], bf16, kind="ExternalInput").ap()
    x_hi = nc.dram_tensor("x_hi", [HALF, D], bf16, kind="ExternalInput").ap()
    xT = nc.dram_tensor("xT", [128, T_CORE * 128], bf16,
                        kind="ExternalInput").ap()
    ce_d = nc.dram_tensor("ce", [128, CE_COLS], bf16,
                          kind="ExternalInput").ap()
    iota_d = nc.dram_tensor("iota", [128, 128], bf16,
                            kind="ExternalInput").ap()
    idx = {}
    for s in dimof:
        idx[s] = nc.dram_tensor(
            f"idx_{s[0]}{s[1]}", [128, NBLK[s] * 8], i16,
            kind="ExternalInput").ap()
    w1 = nc.dram_tensor("W1", [3 * D, D], bf16, kind="ExternalInput").ap()
    wds = {2: nc.dram_tensor("W2", [D, D], bf16, kind="ExternalInput").ap(),
           3: nc.dram_tensor("W3", [D, D], bf16, kind="ExternalInput").ap(),
           4: nc.dram_tensor("W4", [D, D], bf16, kind="ExternalInput").ap()}
    bds = {i: nc.dram_tensor(f"b{i}", [D], f32, kind="ExternalInput").ap()
           for i in (1, 2, 3, 4)}
    out_t = nc.dram_tensor("out_t", [128, T_CORE * 128], f32,
                           kind="ExternalOutput").ap()

    eq = mybir.AluOpType.is_equal
    mul = mybir.AluOpType.mult
    tanh = mybir.ActivationFunctionType.Tanh

    with tile.TileContext(nc) as tc:
        with (
            tc.tile_pool(name="const", bufs=1) as cpool,
            tc.tile_pool(name="gath", bufs=8) as gpool,
            tc.tile_pool(name="idxp", bufs=8) as ipool,
            tc.tile_pool(name="sel", bufs=6) as spool,
            tc.tile_pool(name="hbuf", bufs=3) as hpool,
            tc.tile_pool(name="ps", bufs=4, space="PSUM") as pspool,
            tc.tile_pool(name="psm", bufs=2, space="PSUM") as mpool,
        ):
            iota_t = cpool.tile([128, 128], bf16)
            nc.sync.dma_start(out=iota_t[:], in_=iota_d[:, :])
            cet = cpool.tile([128, CE_COLS], bf16, tag="ce", name="ce")
            nc.sync.dma_start(out=cet[:], in_=ce_d[:, :])
            xt_all = cpool.tile([128, T_CORE * 128], bf16, tag="xt",
                                name="xt")
            nc.sync.dma_start(out=xt_all[:], in_=xT[:, :])
            wt = {}
            for j in range(3):
                wt[(1, j)] = cpool.tile([128, 128], bf16, tag=f"w1{j}",
                                        name=f"w1{j}")
                nc.sync.dma_start(out=wt[(1, j)][:],
                                  in_=w1[j * 128:(j + 1) * 128, :])
            for i in (2, 3, 4):
                wt[i] = cpool.tile([128, 128], bf16, tag=f"w{i}",
                                   name=f"w{i}")
                nc.sync.dma_start(out=wt[i][:], in_=wds[i][:, :])
            bt = {}
            for i in (1, 2, 3, 4):
                bt[i] = cpool.tile([128, 1], f32, tag=f"b{i}", name=f"b{i}")
                nc.sync.dma_start(out=bt[i][:], in_=bds[i][:, None])

            streams = list(dimof.keys())
            chunks = {s: [] for s in streams}
            next_chunk = {s: 0 for s in streams}
            qrr = [0]

            def emit_chunks(upto):
                # upto: dict stream -> block index to cover
                for s in streams:
                    nb_tot = NBLK[s]
                    while (next_chunk[s] * CHUNK < upto[s]
                           and next_chunk[s] * CHUNK < nb_tot):
                        cidx = next_chunk[s]
                        nb = min(CHUNK, nb_tot - cidx * CHUNK)
                        nidx = nb * 128
                        it = ipool.tile([128, nb * 8], i16,
                                        tag=f"i{s[0]}{s[1]}",
                                        name=f"i{s[0]}{s[1]}")
                        nc.sync.dma_start(
                            out=it[:],
                            in_=idx[s][:, cidx * CHUNK * 8:
                                       (cidx * CHUNK + nb) * 8])
                        gb = gpool.tile([128, nb, 128], bf16,
                                        tag=f"g{s[0]}{s[1]}",
                                        name=f"g{s[0]}{s[1]}")
                        q = (qrr[0] + 1) % 4
                        qrr[0] = q
                        nc.gpsimd.dma_gather(
                            out_ap=gb[:],
                            in_ap=(x_lo if s[1] == 0 else x_hi)[:, :],
                            idxs_ap=it[:],
                            num_idxs=nidx,
                            num_idxs_reg=nidx,
                            elem_size=D,
                            single_packet=True,
                            queue_num=q,
                        )
                        chunks[s].append(gb)
                        next_chunk[s] += 1

            for t in range(T_CORE):
                la = min(t + LOOKAHEAD, T_CORE)
                emit_chunks({s: int(blk_start[s][la]) for s in streams})

                acc = {}
                for dname in ("mi", "mo"):
                    nb0 = int(progmax[t, dimof[(dname, 0)]])
                    nb1 = int(progmax[t, dimof[(dname, 1)]])
                    nbt = nb0 + nb1
                    base = int(ce_off[t]) if dname == "mi" else \
                        int(ce_off[t] + 2 * nb_mi[t])
                    ps = pspool.tile([128, 128], f32, tag="scat")
                    for j in range(nbt):
                        h = 0 if j < nb0 else 1
                        jj = j - h * nb0
                        s = (dname, h)
                        s_t = spool.tile([128, 128], bf16, tag="s")
                        nc.vector.tensor_scalar(
                            s_t[:], iota_t[:],
                            cet[:, base + j:base + j + 1],
                            cet[:, base + nbt + j:base + nbt + j + 1],
                            eq, mul)
                        blk = int(blk_start[s][t]) + jj
                        y = chunks[s][blk // CHUNK][:, blk % CHUNK, :]
                        nc.tensor.matmul(
                            out=ps[:], lhsT=y, rhs=s_t[:],
                            start=(j == 0), stop=(j == nbt - 1))
                    acc[dname] = hpool.tile([128, 128], bf16,
                                            tag=f"acc{dname}",
                                            name=f"acc{dname}")
                    nc.scalar.copy(out=acc[dname][:], in_=ps[:])

                hp = mpool.tile([128, 128], f32, tag="mlp")
                nc.tensor.matmul(out=hp[:], lhsT=wt[(1, 0)][:],
                                 rhs=acc["mi"][:], start=True, stop=False)
                nc.tensor.matmul(out=hp[:], lhsT=wt[(1, 1)][:],
                                 rhs=acc["mo"][:], start=False, stop=False)
                nc.tensor.matmul(out=hp[:], lhsT=wt[(1, 2)][:],
                                 rhs=xt_all[:, t * 128:(t + 1) * 128],
                                 start=False, stop=True)
                hprev = hpool.tile([128, 128], bf16, tag="h")
                nc.scalar.activation(hprev[:], hp[:], tanh,
                                     bias=bt[1][:, 0:1])
                for i in (2, 3, 4):
                    hp = mpool.tile([128, 128], f32, tag="mlp")
                    nc.tensor.matmul(out=hp[:], lhsT=wt[i][:],
                                     rhs=hprev[:], start=True, stop=True)
                    if i < 4:
                        hnext = hpool.tile([128, 128], bf16, tag="h")
                    else:
                        hnext = hpool.tile([128, 128], f32, tag="hout",
                                           name="hout")
                    nc.scalar.activation(hnext[:], hp[:], tanh,
                                         bias=bt[i][:, 0:1])
                    hprev = hnext
                nc.sync.dma_start(
                    out=out_t[:, t * 128:(t + 1) * 128], in_=hprev[:])

    nc.compile()
    _NC_CACHE[sig] = nc
    return nc


def kernel(**inputs):
    global LAST_RESULTS
    _register_ntff_hook()
    x = np.asarray(inputs["x"], np.float32)
    e = np.asarray(inputs["e"], np.float32)
    edge_index = np.asarray(inputs["edge_index"])

    per_core, out_col, layout = _preprocess(x, e, edge_index)
    nc = _build_nc(layout)

    shared = {"W1": np.asarray(inputs["W1"],
                               np.float32).astype(ml_dtypes.bfloat16)}
    for i in (2, 3, 4):
        shared[f"W{i}"] = np.asarray(inputs[f"W{i}"],
                                     np.float32).astype(ml_dtypes.bfloat16)
    for i in (1, 2, 3, 4):
        shared[f"b{i}"] = np.asarray(inputs[f"b{i}"], np.float32)

    in_maps = []
    for k in range(N_CORES):
        m = dict(per_core[k])
        m.update(shared)
        in_maps.append(m)

    res = bass_utils.run_bass_kernel_spmd(nc, in_maps,
                                          core_ids=list(range(N_CORES)))
    LAST_RESULTS = res
    big = np.concatenate([np.asarray(res.results[k]["out_t"], np.float32)
                          for k in range(N_CORES)], axis=1)
    out = big.T[out_col[:N]]
    return np.ascontiguousarray(out.astype(np.float32))


# revision 3
# speedup vs baseline: 1.2312x; 1.2312x over previous
"""GNN message-passing kernel v2 — bf16, balanced variable-block cells.

Differences vs baseline kernel.py:
  - All gather tables / gathered rows / one-hot / matmuls / MLP in bf16
    (psum accumulation stays fp32); final layer emits fp32.
  - Host-side 4-dim greedy balancing of nodes -> tiles so each
    (tile, dir, half) cell is ~1020 edges; per-cell block counts are
    variable (mostly 8), cutting gather descriptors ~10% vs uniform H=9.
  - Tiles dealt to cores sorted by block profile; program uses the
    position-wise max profile so one SPMD program fits all cores.
  - ce (one-hot keys + e weights) and xT preloaded to SBUF once.
  - One-hot built with two-scalar tensor_scalar (keeps DVE 2x mode).
"""

import os
import sys

sys.path.insert(0, "/opt/trn_rl_repo")

import numpy as np
import ml_dtypes

from concourse import bass, bacc, mybir, tile
from concourse import bass_utils

N = 50000
E = 800000
D = 128
N_CORES = 8
T_CORE = 49
T_TOT = N_CORES * T_CORE
NPAD = T_TOT * 128
HALF = NPAD // 2
CHUNK = 8            # gather blocks per dma_gather call (single packet)
LOOKAHEAD = 3        # tiles of gather lookahead

f32 = mybir.dt.float32
bf16 = mybir.dt.bfloat16
i16 = mybir.dt.int16

LAST_RESULTS = None


def _register_ntff_hook():
    import types, ctypes, contextlib

    if "antenv.axon_hooks" in sys.modules:
        return
    so_path = "/opt/axon/libaxon_pjrt.so"
    if not os.path.exists(so_path):
        return
    try:
        lib = ctypes.CDLL(so_path)
        if not hasattr(lib, "axon_start_nrt_profile"):
            return
        lib.axon_start_nrt_profile.argtypes = [
            ctypes.POINTER(ctypes.c_int64), ctypes.c_size_t]
        lib.axon_start_nrt_profile.restype = ctypes.c_int64
        lib.axon_stop_nrt_profile.argtypes = [ctypes.c_char_p]
        lib.axon_stop_nrt_profile.restype = ctypes.c_int64

        @contextlib.contextmanager
        def _hook(output_dir, device_ids):
            import jax
            jax.devices()
            if device_ids:
                ids = (ctypes.c_int64 * len(device_ids))(*device_ids)
                rc = lib.axon_start_nrt_profile(ids, len(device_ids))
            else:
                rc = lib.axon_start_nrt_profile(None, 0)
            if rc != 0:
                raise RuntimeError(f"axon_start_nrt_profile rc={rc}")
            try:
                yield
            finally:
                n = lib.axon_stop_nrt_profile(str(output_dir).encode())
                print(f"profile: {n} file(s) -> {output_dir}", file=sys.stderr)

        mod = types.ModuleType("antenv.axon_hooks")
        mod.get_axon_ntff_profile_hook = lambda: _hook
        sys.modules["antenv.axon_hooks"] = mod
    except OSError:
        pass


def _wrap_idx(arr):
    L = arr.shape[0]
    w = arr.reshape(L // 16, 16).T
    return np.ascontiguousarray(np.tile(w, (8, 1)))


def _greedy_tiles(c):
    """Assign padded nodes to (tile, part) balancing the 4 cell loads."""
    tot = c.sum(1)
    order = np.argsort(-tot, kind="stable")
    loads = np.zeros((T_TOT, 4), np.float64)
    counts = np.zeros(T_TOT, np.int64)
    gtile = np.empty(NPAD, np.int32)
    gpart = np.empty(NPAD, np.int32)
    for u in order:
        proj = loads + c[u]
        score = proj.max(1) + 1e-3 * proj.sum(1)
        score[counts >= 128] = 1e18
        t = int(np.argmin(score))
        gtile[u] = t
        gpart[u] = counts[t]
        loads[t] += c[u]
        counts[t] += 1
    return gtile, gpart


def _preprocess(x, e, edge_index):
    src = np.asarray(edge_index[0], np.int64)
    dst = np.asarray(edge_index[1], np.int64)
    ew = np.asarray(e, np.float32)

    c = np.zeros((NPAD, 4), np.int64)
    c[:, 0] = np.bincount(dst[src < HALF], minlength=NPAD)
    c[:, 1] = np.bincount(dst[src >= HALF], minlength=NPAD)
    c[:, 2] = np.bincount(src[dst < HALF], minlength=NPAD)
    c[:, 3] = np.bincount(src[dst >= HALF], minlength=NPAD)
    gtile, gpart = _greedy_tiles(c)

    cells = np.zeros((T_TOT, 4), np.int64)
    cells[:, 0] = np.bincount(gtile[dst[src < HALF]], minlength=T_TOT)
    cells[:, 1] = np.bincount(gtile[dst[src >= HALF]], minlength=T_TOT)
    cells[:, 2] = np.bincount(gtile[src[dst < HALF]], minlength=T_TOT)
    cells[:, 3] = np.bincount(gtile[src[dst >= HALF]], minlength=T_TOT)
    blocks = np.ceil(cells / 128).astype(np.int64)

    # deal tiles to cores (snake by total blocks), sort within core by profile
    key = blocks.sum(1) * 10**9 + blocks[:, 0] * 10**6 + blocks[:, 1] * 10**3
    order = np.argsort(-key, kind="stable")
    core_tiles = [[] for _ in range(N_CORES)]
    for i, t in enumerate(order):
        r, k = divmod(i, N_CORES)
        core_tiles[k if r % 2 == 0 else N_CORES - 1 - k].append(int(t))
    for k in range(N_CORES):
        core_tiles[k].sort(key=lambda t: tuple(-blocks[t]))
    prof = np.array([[blocks[t] for t in core_tiles[k]]
                     for k in range(N_CORES)])          # [8, 49, 4]
    progmax = prof.max(0)                               # [49, 4] program profile

    # per-direction sorted edge partitions by cell = gtile(key)*2 + half(gat)
    def _sort_dir(keyn, gatn):
        cell = gtile[keyn].astype(np.int64) * 2 + (gatn >= HALF)
        o = np.argsort(cell, kind="stable")
        cnt = np.bincount(cell[o], minlength=T_TOT * 2)
        starts = np.zeros(T_TOT * 2 + 1, np.int64)
        starts[1:] = np.cumsum(cnt)
        return o, starts

    o_mi, st_mi = _sort_dir(dst, src)
    o_mo, st_mo = _sort_dir(src, dst)

    # stream sizes (uniform across cores by construction)
    dimof = {("mi", 0): 0, ("mi", 1): 1, ("mo", 0): 2, ("mo", 1): 3}
    NBLK = {s: int(progmax[:, dimof[s]].sum()) for s in dimof}
    # per-tile-position block starts per stream
    blk_start = {s: np.concatenate([[0], np.cumsum(progmax[:, dimof[s]])])
                 for s in dimof}
    # ce columns per tile position: [mi_k | mi_e | mo_k | mo_e]
    nb_mi = progmax[:, 0] + progmax[:, 1]
    nb_mo = progmax[:, 2] + progmax[:, 3]
    ce_w = 2 * (nb_mi + nb_mo)
    ce_off = np.concatenate([[0], np.cumsum(ce_w)])
    CE_COLS = int(ce_off[-1])

    perm_nodes = np.empty(NPAD, np.int64)
    gslot = gtile.astype(np.int64) * 128 + gpart
    perm_nodes[gslot] = np.arange(NPAD)
    xpad = np.zeros((NPAD, D), np.float32)
    xpad[:N] = np.asarray(x, np.float32)
    xpermT = np.ascontiguousarray(xpad[perm_nodes].T)   # [128, NPAD] f32

    per_core = []
    x_lo = np.asarray(xpad[:HALF], ml_dtypes.bfloat16)
    x_hi = np.asarray(xpad[HALF:], ml_dtypes.bfloat16)
    iota = np.broadcast_to(np.arange(128, dtype=np.float32),
                           (128, 128)).astype(ml_dtypes.bfloat16)

    for k in range(N_CORES):
        gidx = {s: np.zeros(NBLK[s] * 128, np.int16) for s in dimof}
        ce = np.zeros((128, CE_COLS), np.float32)
        for pos in range(T_CORE):
            t = core_tiles[k][pos]
            for dname, (ordx, starts, keyn, gatn) in (
                    ("mi", (o_mi, st_mi, dst, src)),
                    ("mo", (o_mo, st_mo, src, dst))):
                base = ce_off[pos] if dname == "mi" else \
                    ce_off[pos] + 2 * nb_mi[pos]
                nbt = nb_mi[pos] if dname == "mi" else nb_mo[pos]
                joff = 0
                for h in (0, 1):
                    s = (dname, h)
                    eidx = ordx[starts[t * 2 + h]:starts[t * 2 + h + 1]]
                    cnt = len(eidx)
                    nb_p = progmax[pos, dimof[s]]
                    assert cnt <= nb_p * 128
                    goff = blk_start[s][pos] * 128
                    gidx[s][goff:goff + cnt] = (
                        gatn[eidx] - h * HALF).astype(np.int16)
                    # key/e columns: block j, edge p -> [p, base + joff + j]
                    kcol = np.zeros(nb_p * 128, np.float32)
                    ecol = np.zeros(nb_p * 128, np.float32)
                    kcol[:cnt] = gpart[keyn[eidx]]
                    ecol[:cnt] = ew[eidx]
                    ce[:, base + joff:base + joff + nb_p] = \
                        kcol.reshape(nb_p, 128).T
                    ce[:, base + nbt + joff:base + nbt + joff + nb_p] = \
                        ecol.reshape(nb_p, 128).T
                    joff += nb_p
        ts = core_tiles[k]
        slot_sel = np.concatenate(
            [np.arange(t * 128, (t + 1) * 128) for t in ts])
        m = {
            "x_lo": x_lo,
            "x_hi": x_hi,
            "xT": np.ascontiguousarray(
                xpermT[:, slot_sel]).astype(ml_dtypes.bfloat16),
            "ce": np.asarray(ce, ml_dtypes.bfloat16),
            "iota": iota,
        }
        for s in dimof:
            m[f"idx_{s[0]}{s[1]}"] = _wrap_idx(gidx[s])
        per_core.append(m)

    layout = {
        "progmax": progmax, "NBLK": NBLK, "blk_start": blk_start,
        "nb_mi": nb_mi, "nb_mo": nb_mo, "ce_off": ce_off,
        "CE_COLS": CE_COLS, "dimof": dimof,
    }
    # output slot of original node n on the PERMUTED (core-dealt) grid:
    # core k, position pos, partition p  ->  global out column
    tile_to_pos = np.empty(T_TOT, np.int64)
    for k in range(N_CORES):
        for pos, t in enumerate(core_tiles[k]):
            tile_to_pos[t] = k * T_CORE + pos
    out_col = tile_to_pos[gtile] * 128 + gpart   # [NPAD]
    return per_core, out_col, layout


_NC_CACHE = {}


def _build_nc(layout):
    sig = (tuple(layout["progmax"].reshape(-1).tolist()),)
    if sig in _NC_CACHE:
        return _NC_CACHE[sig]
    progmax = layout["progmax"]
    dimof = layout["dimof"]
    NBLK = layout["NBLK"]
    blk_start = layout["blk_start"]
    nb_mi = layout["nb_mi"]
    nb_mo = layout["nb_mo"]
    ce_off = layout["ce_off"]
    CE_COLS = layout["CE_COLS"]

    nc = bacc.Bacc("TRN2", target_bir_lowering=False, debug=False,
                   enable_asserts=False, num_devices=N_CORES,
                   num_swdge_queues=4)

    x_lo = nc.dram_tensor("x_lo", [HALF, D], bf16, kind="ExternalInput").ap()
    x_hi = nc.dram_tensor("x_hi", [HALF, D], bf16, kind="ExternalInput").ap()
    xT = nc.dram_tensor("xT", [128, T_CORE * 128], bf16,
                        kind="ExternalInput").ap()
    ce_d = nc.dram_tensor("ce", [128, CE_COLS], bf16,
                          kind="ExternalInput").ap()
    iota_d = nc.dram_tensor("iota", [128, 128], bf16,
                            kind="ExternalInput").ap()
    idx = {}
    for s in dimof:
        idx[s] = nc.dram_tensor(
            f"idx_{s[0]}{s[1]}", [128, NBLK[s] * 8], i16,
            kind="ExternalInput").ap()
    w1 = nc.dram_tensor("W1", [3 * D, D], bf16, kind="ExternalInput").ap()
    wds = {2: nc.dram_tensor("W2", [D, D], bf16, kind="ExternalInput").ap(),
           3: nc.dram_tensor("W3", [D, D], bf16, kind="ExternalInput").ap(),
           4: nc.dram_tensor("W4", [D, D], bf16, kind="ExternalInput").ap()}
    bds = {i: nc.dram_tensor(f"b{i}", [D], f32, kind="ExternalInput").ap()
           for i in (1, 2, 3, 4)}
    out_t = nc.dram_tensor("out_t", [128, T_CORE * 128], bf16,
                           kind="ExternalOutput").ap()

    eq = mybir.AluOpType.is_equal
    mul = mybir.AluOpType.mult
    tanh = mybir.ActivationFunctionType.Tanh

    with tile.TileContext(nc) as tc:
        with (
            tc.tile_pool(name="const", bufs=1) as cpool,
            tc.tile_pool(name="gath", bufs=8) as gpool,
            tc.tile_pool(name="idxp", bufs=8) as ipool,
            tc.tile_pool(name="sel", bufs=6) as spool,
            tc.tile_pool(name="hbuf", bufs=3) as hpool,
            tc.tile_pool(name="ps", bufs=4, space="PSUM") as pspool,
            tc.tile_pool(name="psm", bufs=2, space="PSUM") as mpool,
        ):
            out_sb = cpool.tile([128, T_CORE * 128], bf16, tag="osb",
                                name="osb")
            iota_t = cpool.tile([128, 128], bf16)
            nc.sync.dma_start(out=iota_t[:], in_=iota_d[:, :])
            cet = cpool.tile([128, CE_COLS], bf16, tag="ce", name="ce")
            nc.sync.dma_start(out=cet[:], in_=ce_d[:, :])
            xt_all = cpool.tile([128, T_CORE * 128], bf16, tag="xt",
                                name="xt")
            nc.sync.dma_start(out=xt_all[:], in_=xT[:, :])
            wt = {}
            for j in range(3):
                wt[(1, j)] = cpool.tile([128, 128], bf16, tag=f"w1{j}",
                                        name=f"w1{j}")
                nc.sync.dma_start(out=wt[(1, j)][:],
                                  in_=w1[j * 128:(j + 1) * 128, :])
            for i in (2, 3, 4):
                wt[i] = cpool.tile([128, 128], bf16, tag=f"w{i}",
                                   name=f"w{i}")
                nc.sync.dma_start(out=wt[i][:], in_=wds[i][:, :])
            bt = {}
            for i in (1, 2, 3, 4):
                bt[i] = cpool.tile([128, 1], f32, tag=f"b{i}", name=f"b{i}")
                nc.sync.dma_start(out=bt[i][:], in_=bds[i][:, None])

            streams = list(dimof.keys())
            chunks = {s: [] for s in streams}
            next_chunk = {s: 0 for s in streams}
            qrr = [0]

            def emit_chunks(upto):
                # upto: dict stream -> block index to cover
                for s in streams:
                    nb_tot = NBLK[s]
                    while (next_chunk[s] * CHUNK < upto[s]
                           and next_chunk[s] * CHUNK < nb_tot):
                        cidx = next_chunk[s]
                        nb = min(CHUNK, nb_tot - cidx * CHUNK)
                        nidx = nb * 128
                        it = ipool.tile([128, nb * 8], i16,
                                        tag=f"i{s[0]}{s[1]}",
                                        name=f"i{s[0]}{s[1]}")
                        nc.sync.dma_start(
                            out=it[:],
                            in_=idx[s][:, cidx * CHUNK * 8:
                                       (cidx * CHUNK + nb) * 8])
                        gb = gpool.tile([128, nb, 128], bf16,
                                        tag=f"g{s[0]}{s[1]}",
                                        name=f"g{s[0]}{s[1]}")
                        q = (qrr[0] + 1) % 4
                        qrr[0] = q
                        nc.gpsimd.dma_gather(
                            out_ap=gb[:],
                            in_ap=(x_lo if s[1] == 0 else x_hi)[:, :],
                            idxs_ap=it[:],
                            num_idxs=nidx,
                            num_idxs_reg=nidx,
                            elem_size=D,
                            single_packet=True,
                            queue_num=q,
                        )
                        chunks[s].append(gb)
                        next_chunk[s] += 1

            for t in range(T_CORE):
                la = min(t + LOOKAHEAD, T_CORE)
                emit_chunks({s: int(blk_start[s][la]) for s in streams})

                acc = {}
                for dname in ("mi", "mo"):
                    nb0 = int(progmax[t, dimof[(dname, 0)]])
                    nb1 = int(progmax[t, dimof[(dname, 1)]])
                    nbt = nb0 + nb1
                    base = int(ce_off[t]) if dname == "mi" else \
                        int(ce_off[t] + 2 * nb_mi[t])
                    ps = pspool.tile([128, 128], f32, tag="scat")
                    for j in range(nbt):
                        h = 0 if j < nb0 else 1
                        jj = j - h * nb0
                        s = (dname, h)
                        s_t = spool.tile([128, 128], bf16, tag="s")
                        nc.vector.tensor_scalar(
                            s_t[:], iota_t[:],
                            cet[:, base + j:base + j + 1],
                            cet[:, base + nbt + j:base + nbt + j + 1],
                            eq, mul)
                        blk = int(blk_start[s][t]) + jj
                        y = chunks[s][blk // CHUNK][:, blk % CHUNK, :]
                        nc.tensor.matmul(
                            out=ps[:], lhsT=y, rhs=s_t[:],
                            start=(j == 0), stop=(j == nbt - 1))
                    acc[dname] = hpool.tile([128, 128], bf16,
                                            tag=f"acc{dname}",
                                            name=f"acc{dname}")
                    nc.scalar.copy(out=acc[dname][:], in_=ps[:])

                hp = mpool.tile([128, 128], f32, tag="mlp")
                nc.tensor.matmul(out=hp[:], lhsT=wt[(1, 0)][:],
                                 rhs=acc["mi"][:], start=True, stop=False)
                nc.tensor.matmul(out=hp[:], lhsT=wt[(1, 1)][:],
                                 rhs=acc["mo"][:], start=False, stop=False)
                nc.tensor.matmul(out=hp[:], lhsT=wt[(1, 2)][:],
                                 rhs=xt_all[:, t * 128:(t + 1) * 128],
                                 start=False, stop=True)
                hprev = hpool.tile([128, 128], bf16, tag="h")
                nc.scalar.activation(hprev[:], hp[:], tanh,
                                     bias=bt[1][:, 0:1])
                for i in (2, 3, 4):
                    hp = mpool.tile([128, 128], f32, tag="mlp")
                    nc.tensor.matmul(out=hp[:], lhsT=wt[i][:],
                                     rhs=hprev[:], start=True, stop=True)
                    if i < 4:
                        hnext = hpool.tile([128, 128], bf16, tag="h")
                        nc.scalar.activation(hnext[:], hp[:], tanh,
                                             bias=bt[i][:, 0:1])
                        hprev = hnext
                    else:
                        nc.scalar.activation(
                            out_sb[:, t * 128:(t + 1) * 128], hp[:],
                            tanh, bias=bt[i][:, 0:1])
            nc.sync.dma_start(out=out_t[:, :], in_=out_sb[:])

    nc.compile()
    _NC_CACHE[sig] = nc
    return nc


def kernel(**inputs):
    global LAST_RESULTS
    _register_ntff_hook()
    x = np.asarray(inputs["x"], np.float32)
    e = np.asarray(inputs["e"], np.float32)
    edge_index = np.asarray(inputs["edge_index"])

    per_core, out_col, layout = _preprocess(x, e, edge_index)
    nc = _build_nc(layout)

    shared = {"W1": np.asarray(inputs["W1"],
                               np.float32).astype(ml_dtypes.bfloat16)}
    for i in (2, 3, 4):
        shared[f"W{i}"] = np.asarray(inputs[f"W{i}"],
                                     np.float32).astype(ml_dtypes.bfloat16)
    for i in (1, 2, 3, 4):
        shared[f"b{i}"] = np.asarray(inputs[f"b{i}"], np.float32)

    in_maps = []
    for k in range(N_CORES):
        m = dict(per_core[k])
        m.update(shared)
        in_maps.append(m)

    res = bass_utils.run_bass_kernel_spmd(nc, in_maps,
                                          core_ids=list(range(N_CORES)))
    LAST_RESULTS = res
    big = np.concatenate([np.asarray(res.results[k]["out_t"], np.float32)
                          for k in range(N_CORES)], axis=1)
    out = big.T[out_col[:N]]
    return np.ascontiguousarray(out.astype(np.float32))
